# revision 64
# baseline (speedup 1.0000x reference)
"""Trainium2 Bass kernel for nn_AdditiveCouplingLayer (additive coupling + 5-block
BatchNorm MLP), data-parallel over 8 NeuronCores.

Strategy (v2):
  - Shard batch (16384) across 8 cores (2048 rows each); weights replicated.
  - Activations stay TRANSPOSED on chip: h^T is [hidden, batch] so hidden units
    map to SBUF partitions; BatchNorm stats are free-dim reductions.
  - All matmul operands are bf16 (weights converted on host). This enables the
    PE's Fast-Weight-Load path so LDWEIGHTS overlaps the matmul stream, which
    f32r (4-byte) weights cannot do.
  - Hidden layers loop (m, k, n): one stationary weight tile feeds 4 moving
    chunks of 512 accumulating into 4 PSUM banks, amortizing the weight load.
  - Output stage uses h^T blocks as the STATIONARY operand with Wout moving,
    so y emerges in natural [batch, latent] layout with no PE transposes;
    bout is applied via a K=1 rank-1 matmul that starts each PSUM group.
  - BatchNorm batch stats cross the 8 cores via a small AllGather + local
    reduce; warm-up collectives at kernel start absorb the ncfw cold cost.
  - Next layer's weights are DMA-prefetched during the current layer's
    matmuls; output-stage x tiles prefetch during the last hidden layer.
"""

import sys

sys.path.insert(0, "/opt/trn_rl_repo")

import numpy as np

BN_EPS = 1e-5

# Full-problem constants
B_FULL, D_FULL, H_FULL, NL_FULL, NCORES = 16384, 784, 1024, 5, 8


def build_kernel(B=B_FULL, D=D_FULL, H=H_FULL, NL=NL_FULL, n_cores=NCORES):
    import concourse.bacc as bacc
    import concourse.mybir as mybir
    from concourse import tile, masks

    f32 = mybir.dt.float32
    bf16 = mybir.dt.bfloat16
    AF = mybir.ActivationFunctionType
    ALU = mybir.AluOpType
    AX = mybir.AxisListType

    L = D // 2                     # latent width (coupling half)
    C = B // n_cores               # batch rows per core
    LT = (L + 127) // 128          # latent k-tiles (win padded to LT*128 rows)
    LREM = L - (LT - 1) * 128      # real rows in last latent tile
    MT = H // 128                  # hidden m/k tiles
    NCHW = 512                     # moving free-dim chunk
    NCH = C // NCHW                # chunks per row-block
    BPC = NCHW // 128              # batch tiles per chunk

    nc = bacc.Bacc("TRN2", target_bir_lowering=False, debug=False,
                   num_devices=n_cores)

    x_d = nc.dram_tensor("x", [C, D], f32, kind="ExternalInput")
    win_d = nc.dram_tensor("win", [LT * 128, H], bf16, kind="ExternalInput")
    wh_d = nc.dram_tensor("wh", [NL, H, H], bf16, kind="ExternalInput")
    wout_d = nc.dram_tensor("wout", [H, L], bf16, kind="ExternalInput")
    bin_d = nc.dram_tensor("bin", [H], f32, kind="ExternalInput")
    bh_d = nc.dram_tensor("bh", [NL, H], f32, kind="ExternalInput")
    gamma_d = nc.dram_tensor("gamma", [NL, H], f32, kind="ExternalInput")
    beta_d = nc.dram_tensor("beta", [NL, H], f32, kind="ExternalInput")
    bout_d = nc.dram_tensor("bout", [1, L], bf16, kind="ExternalInput")
    out_d = nc.dram_tensor("out", [C, D], f32, kind="ExternalOutput")

    rg = [list(range(n_cores))]

    with tile.TileContext(nc) as tc:
        with (
            tc.tile_pool(name="w", bufs=2) as wp,
            tc.tile_pool(name="h", bufs=1) as hp,
            tc.tile_pool(name="xio", bufs=2) as xp,
            tc.tile_pool(name="small", bufs=2) as sp,
            tc.tile_pool(name="psum", bufs=6, space="PSUM") as pp,
            tc.tile_pool(name="dram", bufs=2, space="DRAM") as dp,
            tc.tile_pool(name="const", bufs=1) as cp,
        ):
            ident = cp.tile([128, 128], bf16)
            masks.make_identity(nc, ident[:])
            ones = cp.tile([1, 128], bf16)
            nc.gpsimd.memset(ones[:], 1.0)
            zrow = cp.tile([128, NCHW], bf16)
            nc.vector.memset(zrow[:], 0.0)
            eps_t = cp.tile([128, 1], f32)
            nc.vector.memset(eps_t[:], BN_EPS)
            ones_f = cp.tile([128, 8], f32)
            nc.vector.memset(ones_f[:], 1.0)
            # ~4us of dummy matmuls keeps the PE HAM busy during the DMA
            # lead-in so real work starts at full clock
            for wu in range(10):
                psw = pp.tile([128, NCHW], f32, tag="mm", name=f"warmmm{wu}")
                nc.tensor.matmul(psw[:], zrow[:, 0:128], zrow[:])

            # Warm-up collectives: absorb the first-collective ncfw cold start
            # on the CC cores, fully overlapped with the input stage.
            zf = cp.tile([128, 16], f32)
            nc.vector.memset(zf[:], 0.0)
            for wu in range(2):
                # one warm-up per real payload shape.  No more: warm-ups
                # serialize at ~9us each on the CC engine, and a longer
                # chain backlogs layer 0's real AllGather behind them.
                wref = 10 if wu % 2 == 0 else 6
                wtag = "1" if wu % 2 == 0 else "2"
                warm_in = dp.tile([128, wref], f32, tag=f"arin{wtag}",
                                  name=f"warmin{wu}")
                warm_out = dp.tile([n_cores * 128, wref], f32,
                                   tag=f"arout{wtag}",
                                   name=f"warmout{wu}", addr_space="Shared")
                nc.gpsimd.dma_start(warm_in[:], zf[:, 0:wref])
                nc.gpsimd.collective_compute(
                    "AllGather", ALU.bypass, replica_groups=rg,
                    ins=[warm_in.opt()], outs=[warm_out.opt()])

            # ---- Stage A interleaved per chunk:
            #   x1^T = transpose(x[:, 0::2]);  h0^T = Win^T @ x1^T + bin ----
            win = [wp.tile([128, H], bf16, tag=f"w{k}", name=f"wtin_{k}")
                   for k in range(LT)]
            binT = sp.tile([128, MT], f32, tag="biasT")

            x1 = [hp.tile([128, C], bf16, tag=f"xt{j}", name=f"x1_{j}")
                  for j in range(LT)]
            if LREM < 128:
                # zero the padded partitions once (whole tile: partition
                # slices must be 32-aligned; real rows overwritten below).
                # On DVE: the GpSimd queue is serialized behind the warmup
                # collectives' pool-rotation waits and would gate stage A.
                nc.vector.memset(x1[LT - 1][:], 0.0)
            cur = [hp.tile([128, C], bf16, tag=f"hg{m}", name=f"h0_{m}")
                   for m in range(MT)]
            hblT = sp.tile([128, MT], f32, tag="biasL", name="bh_l0")
            gT = sp.tile([128, MT], f32, tag="gT", name="g_l0")
            bT = sp.tile([128, MT], f32, tag="bT", name="b_l0")
            for n in range(NCH):
                ncs = slice(n * NCHW, (n + 1) * NCHW)
                for b in range(n * BPC, (n + 1) * BPC):
                    xin = xp.tile([128, D], f32, tag="xin", bufs=5)
                    if b == 0:
                        # split the first tile across 4 HWDGE queues so the
                        # input chain starts earlier
                        for q in range(4):
                            nc.sync.dma_start(
                                xin[q * 32:(q + 1) * 32, :],
                                x_d[b * 128 + q * 32:b * 128 + (q + 1) * 32, :])
                    else:
                        nc.sync.dma_start(xin[:], x_d[b * 128:(b + 1) * 128, :])
                    xe = xp.tile([128, L], bf16, tag="xe", bufs=3)
                    nc.scalar.copy(
                        xe[:],
                        xin[:].rearrange("p (l two) -> p l two", two=2)[:, :, 0])
                    for j in range(LT):
                        wj = 128 if j < LT - 1 else LREM
                        ps = pp.tile([128, 128], bf16, tag="tr", bufs=2)
                        nc.tensor.transpose(ps[0:wj, :],
                                            xe[:, j * 128:j * 128 + wj],
                                            ident[:])
                        nc.vector.tensor_copy(
                            x1[j][0:wj, b * 128:(b + 1) * 128], ps[0:wj, :])
                if n == 0:
                    # Win/bin DMAs issued after chunk-0's x tiles so the
                    # input chain isn't stuck behind weight traffic
                    for k in range(LT):
                        nc.sync.dma_start(win[k][:],
                                          win_d[k * 128:(k + 1) * 128, :])
                    nc.sync.dma_start(
                        binT[:], bin_d[:].rearrange("(m p) -> p m", p=128))
                if n == NCH - 1:
                    # layer 0's weights + bn params, after the last x tiles
                    # so they don't delay the stage-A input pipeline
                    nc.sync.dma_start(hblT[:],
                                      bh_d[0, :].rearrange("(m p) -> p m",
                                                           p=128))
                    nc.sync.dma_start(gT[:],
                                      gamma_d[0, :].rearrange("(m p) -> p m",
                                                              p=128))
                    nc.sync.dma_start(bT[:],
                                      beta_d[0, :].rearrange("(m p) -> p m",
                                                             p=128))
                for m in range(MT):
                    ps = pp.tile([128, NCHW], f32, tag="mm")
                    for k in range(LT):
                        nc.tensor.matmul(ps[:], win[k][:, m * 128:(m + 1) * 128],
                                         x1[k][:, ncs],
                                         start=(k == 0), stop=(k == LT - 1))
                    # bias-add drain alternates DVE/ACT so PSUM slots recycle
                    # at PE rate; output is the bf16 moving operand of layer 0
                    if m % 2 == 0:
                        nc.vector.tensor_scalar(
                            out=cur[m][:, ncs], in0=ps[:],
                            scalar1=binT[:, m:m + 1], scalar2=None, op0=ALU.add)
                    else:
                        nc.scalar.activation(
                            cur[m][:, ncs], ps[:], AF.Identity,
                            bias=binT[:, m:m + 1], scale=1.0)
                if n == NCH - 1:
                    wt = [wp.tile([128, H], bf16, tag=f"w{k}",
                                  name=f"wh_0_{k}") for k in range(MT)]
                    for k in range(MT):
                        nc.sync.dma_start(wt[k][:],
                                          wh_d[0, k * 128:(k + 1) * 128, :])

            # ---- Hidden blocks: h = BN(relu(Wh^T @ h + bh)) ----
            # The per-layer BN sync is software-pipelined around the ~12us
            # AllGather flight.  Stats for m=0..4 ship in AG#1 (launched
            # mid-m-loop, landing before the loop ends); m=5..7 in AG#2
            # (launched right after m=7's stats).  The consumer hides AG#2's
            # flight with deferred-k accumulation: the first SPILL_M m-tiles
            # accumulate k=0..4 into PSUM, spill the partial to SBUF (freeing
            # the bank), and only later add k=5..7 once AG#2's normalize is
            # out.  That yields ~21us of PE work that needs no AG#2 data.
            MSPLIT = 5
            SPILL_M = 4

            def bn_launch(tag, sl, mean_ap, var_ap, l):
                # pack local (mean, var) -> (sum, sumsq)/B; AllReduce sums
                # them across cores.  (AllGather + on-chip reduce needs a
                # strided 56B-segment gather-back DMA that costs ~6us.)
                w = sl.stop - sl.start
                sums = sp.tile([128, 2 * w], f32, tag=f"sums{tag}",
                               name=f"sums{tag}_{l}")
                nc.vector.tensor_scalar_mul(sums[:, 0:w], mean_ap[:, sl],
                                            float(C) / B)
                msq = sp.tile([128, w], f32, tag=f"msq{tag}",
                               name=f"msq{tag}_{l}")
                nc.vector.tensor_mul(msq[:], mean_ap[:, sl], mean_ap[:, sl])
                nc.vector.tensor_add(sums[:, w:2 * w], var_ap[:, sl], msq[:])
                nc.vector.tensor_scalar_mul(sums[:, w:2 * w],
                                            sums[:, w:2 * w], float(C) / B)
                agin = dp.tile([128, 2 * w], f32, tag=f"arin{tag}",
                               name=f"arin{tag}_{l}")
                agout = dp.tile([n_cores * 128, 2 * w], f32, tag=f"arout{tag}",
                                name=f"arout{tag}_{l}", addr_space="Shared")
                nc.gpsimd.dma_start(agin[:], sums[:])
                # AllGather, not AllReduce: the ring AR costs ~2x the steps
                # (~19us vs ~6us measured for this tiny payload)
                nc.gpsimd.collective_compute(
                    "AllGather", ALU.bypass, replica_groups=rg,
                    ins=[agin.opt()], outs=[agout.opt()])
                return agout

            def bn_gather(tag, agout, sl, l):
                # gather-back is DMA-segment-rate bound (128 tiny segments
                # per rank): split per-rank over GpSimd+scalar queues.
                # Emitted only where those queues have no pending drain
                # work, since the DMAs wait on the collective.
                w = sl.stop - sl.start
                gall = sp.tile([128, n_cores * 2 * w], f32, tag=f"gall{tag}",
                               name=f"gall{tag}_{l}")
                engs = [nc.gpsimd, nc.scalar, nc.sync]
                for r in range(n_cores):
                    engs[r % 3].dma_start(
                        gall[:, r * 2 * w:(r + 1) * 2 * w],
                        agout[r * 128:(r + 1) * 128, :])
                return gall

            def bn_finish(tag, sl, gall, gT, bT, aa, bb, l, eng):
                # reduce over ranks, then a = gamma*rsqrt(var+eps),
                # b = beta - mean*a.  `eng` picks the vector engine so this
                # chain stays off the queue that m=7's stats and AG#2's
                # pack need.
                w = sl.stop - sl.start
                gst = sp.tile([128, 2 * w], f32, tag=f"gst{tag}",
                              name=f"gst{tag}_{l}")
                if eng is nc.gpsimd:
                    # pairwise-add tree on GpSimd: the DVE-only tensor_reduce
                    # would queue behind m7's drain+stats pipeline (~6us)
                    S = 2 * w
                    t4 = sp.tile([128, 4 * S], f32, tag=f"t4{tag}",
                                 name=f"t4{tag}_{l}")
                    nc.gpsimd.tensor_add(t4[:], gall[:, 0:4 * S],
                                         gall[:, 4 * S:8 * S])
                    t2 = sp.tile([128, 2 * S], f32, tag=f"t2{tag}",
                                 name=f"t2{tag}_{l}")
                    nc.gpsimd.tensor_add(t2[:], t4[:, 0:2 * S],
                                         t4[:, 2 * S:4 * S])
                    nc.gpsimd.tensor_add(gst[:], t2[:, 0:S], t2[:, S:2 * S])
                else:
                    nc.vector.tensor_reduce(
                        gst[:],
                        gall[:].rearrange("p (r s) -> p s r", s=2 * w),
                        axis=AX.X, op=ALU.add)
                gm = gst[:, 0:w]
                ge2 = gst[:, w:2 * w]
                gve = sp.tile([128, w], f32, tag=f"gve{tag}",
                              name=f"gve{tag}_{l}")
                eng.tensor_mul(gve[:], gm, gm)
                eng.tensor_sub(gve[:], ge2, gve[:])
                gstd = sp.tile([128, w], f32, tag=f"gstd{tag}",
                               name=f"gstd{tag}_{l}")
                nc.scalar.activation(gstd[:], gve[:], AF.Sqrt,
                                     bias=eps_t[:, 0:1], scale=1.0)
                ginv = sp.tile([128, w], f32, tag=f"ginv{tag}",
                               name=f"ginv{tag}_{l}")
                nc.vector.reciprocal(ginv[:], gstd[:])
                eng.tensor_mul(aa[:], gT[:, sl], ginv[:])
                mb = sp.tile([128, w], f32, tag=f"mb{tag}",
                             name=f"mb{tag}_{l}")
                eng.tensor_mul(mb[:], gm, aa[:])
                eng.tensor_sub(bb[:], bT[:, sl], mb[:])

            SL1 = slice(0, MSPLIT)
            SL2 = slice(MSPLIT, MT)
            wo = None
            boutrow = None
            xin2 = [None] * (C // 128)
            for l in range(NL):
                bhT = hblT
                outt = [hp.tile([128, C], bf16, tag=f"hr{m}",
                                name=f"hp_{l}_{m}") for m in range(MT)]
                ag = sp.tile([128, 2 * MT], f32, tag="ag")
                mean_ap = ag[:].rearrange("p (m two) -> p m two", two=2)[:, :, 0]
                var_ap = ag[:].rearrange("p (m two) -> p m two", two=2)[:, :, 1]
                aa1 = sp.tile([128, MSPLIT], f32, tag="aa1")
                bb1 = sp.tile([128, MSPLIT], f32, tag="bb1")
                aa2 = sp.tile([128, MT - MSPLIT], f32, tag="aa2")
                bb2 = sp.tile([128, MT - MSPLIT], f32, tag="bb2")

                # prefetches for the next stage ride the sync queue early
                if l < NL - 1:
                    wtn = [wp.tile([128, H], bf16, tag=f"w{k}",
                                   name=f"wh_{l + 1}_{k}")
                           for k in range(MT)]
                    for k in range(MT):
                        nc.sync.dma_start(
                            wtn[k][:],
                            wh_d[l + 1, k * 128:(k + 1) * 128, :])
                    hblT = sp.tile([128, MT], f32, tag="biasL",
                                   name=f"bh_l{l + 1}")
                    nc.sync.dma_start(
                        hblT[:],
                        bh_d[l + 1, :].rearrange("(m p) -> p m", p=128))
                    gTn = sp.tile([128, MT], f32, tag="gT",
                                  name=f"g_l{l + 1}")
                    nc.sync.dma_start(
                        gTn[:],
                        gamma_d[l + 1, :].rearrange("(m p) -> p m", p=128))
                    bTn = sp.tile([128, MT], f32, tag="bT",
                                  name=f"b_l{l + 1}")
                    nc.sync.dma_start(
                        bTn[:],
                        beta_d[l + 1, :].rearrange("(m p) -> p m", p=128))
                else:
                    wo = [wp.tile([128, L], bf16, tag=f"wo{k}",
                                  bufs=1, name=f"wtout_{k}")
                          for k in range(MT)]
                    for k in range(MT):
                        nc.sync.dma_start(
                            wo[k][:], wout_d[k * 128:(k + 1) * 128, :])
                    boutrow = sp.tile([1, L], bf16, tag="boutT")
                    nc.sync.dma_start(boutrow[:], bout_d[:, :])
                    for b in range(C // 128):
                        xin2[b] = xp.tile([128, D], f32, tag="xin2",
                                          bufs=8, name=f"xin2_{b}")
                        nc.sync.dma_start(
                            xin2[b][:], x_d[b * 128:(b + 1) * 128, :])

                def mm_block(m, k0, k1, l=l):
                    # one accumulation group over k-tiles [k0, k1) x 4 chunks
                    pss = [pp.tile([128, NCHW], f32, tag="mm",
                                   name=f"ps_{l}_{m}_{k0}_{n}")
                           for n in range(NCH)]
                    for k in range(k0, k1):
                        for n in range(NCH):
                            nc.tensor.matmul(
                                pss[n][:], wt[k][:, m * 128:(m + 1) * 128],
                                cur[k][:, n * NCHW:(n + 1) * NCHW],
                                start=(k == k0), stop=(k == k1 - 1))
                    return pss

                def stats_block(m, l=l):
                    st = sp.tile([128, 6 * NCH], f32, tag="st")
                    for n in range(NCH):
                        ncs = slice(n * NCHW, (n + 1) * NCHW)
                        nc.vector.bn_stats(st[:, 6 * n:6 * n + 6],
                                           outt[m][:, ncs])
                    nc.vector.bn_aggr(ag[:, 2 * m:2 * m + 2], st[:])

                spill_m = SPILL_M if l > 0 else 0
                scr = [hp.tile([128, C], bf16,
                               tag=(f"xt{m}" if m < 4 else f"sc{m}"),
                               name=f"sc_{l}_{m}") for m in range(spill_m)]
                # phase A: k=0..MSPLIT-1 accumulate, spill partial to SBUF
                for m in range(spill_m):
                    pss = mm_block(m, 0, MSPLIT)
                    for n in range(NCH):
                        ncs = slice(n * NCHW, (n + 1) * NCHW)
                        nc.vector.tensor_copy(scr[m][:, ncs], pss[n][:])
                # phase B: re-inject the spilled partial through the PE (an
                # identity matmul seeds the accumulation — keeps the merge
                # off DVE, whose stats work would otherwise pace the PE),
                # then k=MSPLIT..MT-1 accumulate and a normal relu drain.
                for m in range(spill_m):
                    pss = [pp.tile([128, NCHW], f32, tag="mm",
                                   name=f"psb_{l}_{m}_{n}")
                           for n in range(NCH)]
                    for n in range(NCH):
                        ncs = slice(n * NCHW, (n + 1) * NCHW)
                        nc.tensor.matmul(pss[n][:], ident[:],
                                         scr[m][:, ncs],
                                         start=True, stop=False)
                    for k in range(MSPLIT, MT):
                        for n in range(NCH):
                            nc.tensor.matmul(
                                pss[n][:], wt[k][:, m * 128:(m + 1) * 128],
                                cur[k][:, n * NCHW:(n + 1) * NCHW],
                                start=False, stop=(k == MT - 1))
                    for n in range(NCH):
                        ncs = slice(n * NCHW, (n + 1) * NCHW)
                        nc.scalar.activation(outt[m][:, ncs], pss[n][:],
                                             AF.Relu, bias=bhT[:, m:m + 1],
                                             scale=1.0)
                    stats_block(m)
                    if m == MSPLIT - 1:
                        # AG#1 (m0..3) launches as soon as its stats exist
                        ag1out = bn_launch("1", SL1, mean_ap, var_ap, l)
                # remaining m-tiles: single full-k accumulation
                for m in range(spill_m, MT):
                    pss = mm_block(m, 0, MT)
                    for n in range(NCH):
                        ncs = slice(n * NCHW, (n + 1) * NCHW)
                        if m == MT - 1:
                            # last tile drains on DVE so the ACT queue is
                            # free the moment the m-loop ends (it holds the
                            # early normalize chunks and finish#1's sqrt)
                            nc.vector.tensor_scalar(
                                out=outt[m][:, ncs], in0=pss[n][:],
                                scalar1=bhT[:, m:m + 1], scalar2=0.0,
                                op0=ALU.add, op1=ALU.max)
                        else:
                            nc.scalar.activation(outt[m][:, ncs], pss[n][:],
                                                 AF.Relu, bias=bhT[:, m:m + 1],
                                                 scale=1.0)
                    stats_block(m)
                    if m == MSPLIT - 1:
                        # AG#1 (m0..4) flies while m5..7 compute
                        ag1out = bn_launch("1", SL1, mean_ap, var_ap, l)
                    if m == MT - 2:
                        # gather + a,b chain emitted here: they run in the
                        # queue-idle window before the m-loop ends (m7's
                        # drains are on DVE; ACT is free after m6's)
                        gall1 = bn_gather("1", ag1out, SL1, l)
                        bn_finish("1", SL1, gall1, gT, bT, aa1, bb1, l,
                                  nc.gpsimd)

                def norm_chunk(k, n, ei, nxt):
                    ncs = slice(n * NCHW, (n + 1) * NCHW)
                    aa, bb, j = ((aa1, bb1, k) if k < MSPLIT
                                 else (aa2, bb2, k - MSPLIT))
                    if ei == 0:
                        nc.vector.tensor_scalar(
                            out=nxt[k][:, ncs], in0=outt[k][:, ncs],
                            scalar1=aa[:, j:j + 1], scalar2=bb[:, j:j + 1],
                            op0=ALU.mult, op1=ALU.add)
                    elif ei == 1:
                        nc.scalar.activation(
                            nxt[k][:, ncs], outt[k][:, ncs], AF.Identity,
                            bias=bb[:, j:j + 1], scale=aa[:, j:j + 1])
                    else:
                        nc.gpsimd.tensor_scalar(
                            out=nxt[k][:, ncs], in0=outt[k][:, ncs],
                            scalar1=aa[:, j:j + 1], scalar2=bb[:, j:j + 1],
                            op0=ALU.mult, op1=ALU.add)

                # normalize k=0..4 immediately (a,b landed mid-loop).
                # Hidden layers consume whole k-rows (k-major); the output
                # stage consumes column-blocks (n-major).  Lead with
                # GpSimd/ACT — DVE still holds m7's drains+stats.
                nxt = [hp.tile([128, C], bf16, tag=f"hg{m}",
                               name=f"hn_{l}_{m}") for m in range(MT)]
                if l < NL - 1:
                    order = [(k, n) for k in range(MSPLIT)
                             for n in range(NCH)]
                else:
                    order = [(k, n) for n in range(NCH)
                             for k in range(MSPLIT)]
                for idx, (k, n) in enumerate(order):
                    ei = (2 if idx % 2 == 0 else 1) if idx < 8 else idx % 3
                    norm_chunk(k, n, ei, nxt)
                # AG#2 (m5..7): pack rides DVE after m7's stats; its tail
                # lands under the next consumer's deferred-k cover
                ag2out = bn_launch("2", SL2, mean_ap, var_ap, l)
                gall2 = bn_gather("2", ag2out, SL2, l)
                bn_finish("2", SL2, gall2, gT, bT, aa2, bb2, l, nc.vector)
                ei = 0
                for k in range(MSPLIT, MT):
                    for n in range(NCH):
                        norm_chunk(k, n, ei, nxt)
                        ei = (ei + 1) % 3
                cur = nxt
                if l < NL - 1:
                    wt = wtn
                    gT = gTn
                    bT = bTn

            # ---- Output stage:  y = h^T-blocks (stationary) @ Wout + bout;
            #   out[:,0::2] = x1, out[:,1::2] = x2 + y  (no PE transposes).
            # Same A/B deferral: every block's bout + k0..4 partial runs
            # while layer 4's AG#2 is in flight, spilled to SBUF; k5..7 and
            # the assembly follow once the last normalize lands. ----
            scro = []
            for b in range(C // 128):
                bs = slice(b * 128, (b + 1) * 128)
                ps = pp.tile([128, NCHW], f32, tag="mm", name=f"oA_{b}")
                # rank-1: seed every row of the PSUM group with bout
                nc.tensor.matmul(ps[:, 0:L], ones[:], boutrow[:],
                                 start=True, stop=False)
                for k in range(MSPLIT):
                    nc.tensor.matmul(ps[:, 0:L], cur[k][:, bs], wo[k][:],
                                     start=False, stop=(k == MSPLIT - 1))
                so = xp.tile([128, L], bf16, tag="so", bufs=16,
                             name=f"so_{b}")
                nc.vector.tensor_copy(so[:], ps[:, 0:L])
                scro.append(so)
            for b in range(C // 128):
                bs = slice(b * 128, (b + 1) * 128)
                ps = pp.tile([128, NCHW], f32, tag="mm", name=f"oB_{b}")
                # identity matmul re-injects the spilled partial, then k5..7
                nc.tensor.matmul(ps[:, 0:L], ident[:], scro[b][:],
                                 start=True, stop=False)
                for k in range(MSPLIT, MT):
                    nc.tensor.matmul(ps[:, 0:L], cur[k][:, bs], wo[k][:],
                                     start=False, stop=(k == MT - 1))
                xo = xp.tile([128, D], f32, tag="xo", bufs=4)
                xin_il = xin2[b][:].rearrange("p (l two) -> p l two", two=2)
                xo_il = xo[:].rearrange("p (l two) -> p l two", two=2)
                nc.scalar.copy(xo_il[:, :, 0], xin_il[:, :, 0])
                # balance the tail pipeline: odd-add alternates DVE/GpSimd
                # (GpSimd can't read PSUM, so it gets a bounce via DVE),
                # output DMA alternates sync/scalar queues
                nc.vector.tensor_add(xo_il[:, :, 1], ps[:, 0:L],
                                     xin_il[:, :, 1])
                if b % 2 == 0:
                    nc.sync.dma_start(out_d[bs, :], xo[:])
                else:
                    nc.scalar.dma_start(out_d[bs, :], xo[:])

    nc.compile()
    return nc


def make_in_maps(x, Win, bin_, Wh, bh, gamma, beta, Wout, bout,
                 B=B_FULL, D=D_FULL, H=H_FULL, n_cores=NCORES):
    import ml_dtypes

    bf16 = ml_dtypes.bfloat16
    L = D // 2
    C = B // n_cores
    LT = (L + 127) // 128
    x = np.ascontiguousarray(np.asarray(x, dtype=np.float32))
    win_p = np.zeros((LT * 128, H), dtype=bf16)
    win_p[:L] = np.asarray(Win, dtype=np.float32).astype(bf16)
    common = {
        "win": win_p,
        "wh": np.ascontiguousarray(np.asarray(Wh, dtype=np.float32)
                                   .astype(bf16)),
        "wout": np.ascontiguousarray(np.asarray(Wout, dtype=np.float32)
                                     .astype(bf16)),
        "bin": np.asarray(bin_, dtype=np.float32),
        "bh": np.ascontiguousarray(np.asarray(bh, dtype=np.float32)),
        "gamma": np.ascontiguousarray(np.asarray(gamma, dtype=np.float32)),
        "beta": np.ascontiguousarray(np.asarray(beta, dtype=np.float32)),
        "bout": np.asarray(bout, dtype=np.float32).astype(bf16)
                .reshape(1, L),
    }
    return [
        {"x": np.ascontiguousarray(x[c * C:(c + 1) * C]), **common}
        for c in range(n_cores)
    ]


_built = None


def kernel(x, Win, bin_, Wh, bh, gamma, beta, Wout, bout):
    global _built
    from concourse.bass_utils import run_bass_kernel_spmd

    if _built is None:
        _built = build_kernel()
    in_maps = make_in_maps(x, Win, bin_, Wh, bh, gamma, beta, Wout, bout)
    res = run_bass_kernel_spmd(_built, in_maps, core_ids=list(range(NCORES)))
    return np.concatenate([r["out"] for r in res.results], axis=0)


# revision 65
# speedup vs baseline: 1.0371x; 1.0371x over previous
"""Trainium2 Bass kernel for nn_AdditiveCouplingLayer (additive coupling + 5-block
BatchNorm MLP), data-parallel over 8 NeuronCores.

Strategy (v2):
  - Shard batch (16384) across 8 cores (2048 rows each); weights replicated.
  - Activations stay TRANSPOSED on chip: h^T is [hidden, batch] so hidden units
    map to SBUF partitions; BatchNorm stats are free-dim reductions.
  - All matmul operands are bf16 (weights converted on host). This enables the
    PE's Fast-Weight-Load path so LDWEIGHTS overlaps the matmul stream, which
    f32r (4-byte) weights cannot do.
  - Hidden layers loop (m, k, n): one stationary weight tile feeds 4 moving
    chunks of 512 accumulating into 4 PSUM banks, amortizing the weight load.
  - Output stage uses h^T blocks as the STATIONARY operand with Wout moving,
    so y emerges in natural [batch, latent] layout with no PE transposes;
    bout is applied via a K=1 rank-1 matmul that starts each PSUM group.
  - BatchNorm batch stats cross the 8 cores via a small AllGather + local
    reduce; warm-up collectives at kernel start absorb the ncfw cold cost.
  - Next layer's weights are DMA-prefetched during the current layer's
    matmuls; output-stage x tiles prefetch during the last hidden layer.
"""

import sys

sys.path.insert(0, "/opt/trn_rl_repo")

import numpy as np

BN_EPS = 1e-5

# Full-problem constants
B_FULL, D_FULL, H_FULL, NL_FULL, NCORES = 16384, 784, 1024, 5, 8


def build_kernel(B=B_FULL, D=D_FULL, H=H_FULL, NL=NL_FULL, n_cores=NCORES):
    import concourse.bacc as bacc
    import concourse.mybir as mybir
    from concourse import tile, masks

    f32 = mybir.dt.float32
    bf16 = mybir.dt.bfloat16
    AF = mybir.ActivationFunctionType
    ALU = mybir.AluOpType
    AX = mybir.AxisListType

    L = D // 2                     # latent width (coupling half)
    C = B // n_cores               # batch rows per core
    LT = (L + 127) // 128          # latent k-tiles (win padded to LT*128 rows)
    LREM = L - (LT - 1) * 128      # real rows in last latent tile
    MT = H // 128                  # hidden m/k tiles
    NCHW = 512                     # moving free-dim chunk
    NCH = C // NCHW                # chunks per row-block
    BPC = NCHW // 128              # batch tiles per chunk

    nc = bacc.Bacc("TRN2", target_bir_lowering=False, debug=False,
                   num_devices=n_cores)

    x_d = nc.dram_tensor("x", [C, D], f32, kind="ExternalInput")
    win_d = nc.dram_tensor("win", [LT * 128, H], bf16, kind="ExternalInput")
    wh_d = nc.dram_tensor("wh", [NL, H, H], bf16, kind="ExternalInput")
    wout_d = nc.dram_tensor("wout", [H, L], bf16, kind="ExternalInput")
    bin_d = nc.dram_tensor("bin", [H], f32, kind="ExternalInput")
    bh_d = nc.dram_tensor("bh", [NL, H], f32, kind="ExternalInput")
    gamma_d = nc.dram_tensor("gamma", [NL, H], f32, kind="ExternalInput")
    beta_d = nc.dram_tensor("beta", [NL, H], f32, kind="ExternalInput")
    bout_d = nc.dram_tensor("bout", [1, L], bf16, kind="ExternalInput")
    out_d = nc.dram_tensor("out", [C, D], f32, kind="ExternalOutput")

    rg = [list(range(n_cores))]

    with tile.TileContext(nc) as tc:
        with (
            tc.tile_pool(name="w", bufs=2) as wp,
            tc.tile_pool(name="h", bufs=1) as hp,
            tc.tile_pool(name="xio", bufs=2) as xp,
            tc.tile_pool(name="small", bufs=2) as sp,
            tc.tile_pool(name="psum", bufs=6, space="PSUM") as pp,
            tc.tile_pool(name="dram", bufs=2, space="DRAM") as dp,
            tc.tile_pool(name="const", bufs=1) as cp,
        ):
            ident = cp.tile([128, 128], bf16)
            masks.make_identity(nc, ident[:])
            ones = cp.tile([1, 128], bf16)
            nc.gpsimd.memset(ones[:], 1.0)
            zrow = cp.tile([128, NCHW], bf16)
            nc.vector.memset(zrow[:], 0.0)
            eps_t = cp.tile([128, 1], f32)
            nc.vector.memset(eps_t[:], BN_EPS)
            ones_f = cp.tile([128, 8], f32)
            nc.vector.memset(ones_f[:], 1.0)
            # ~4us of dummy matmuls keeps the PE HAM busy during the DMA
            # lead-in so real work starts at full clock
            for wu in range(10):
                psw = pp.tile([128, NCHW], f32, tag="mm", name=f"warmmm{wu}")
                nc.tensor.matmul(psw[:], zrow[:, 0:128], zrow[:])

            # Warm-up collectives: absorb the first-collective ncfw cold start
            # on the CC cores, fully overlapped with the input stage.
            zf = cp.tile([128, 16], f32)
            nc.vector.memset(zf[:], 0.0)
            for wu in range(2):
                # one warm-up per real payload shape.  No more: warm-ups
                # serialize at ~9us each on the CC engine, and a longer
                # chain backlogs layer 0's real AllGather behind them.
                wref = 10 if wu % 2 == 0 else 6
                wtag = "1" if wu % 2 == 0 else "2"
                warm_in = dp.tile([128, wref], f32, tag=f"arin{wtag}",
                                  name=f"warmin{wu}")
                warm_out = dp.tile([n_cores * 128, wref], f32,
                                   tag=f"arout{wtag}",
                                   name=f"warmout{wu}", addr_space="Shared")
                nc.gpsimd.dma_start(warm_in[:], zf[:, 0:wref])
                nc.gpsimd.collective_compute(
                    "AllGather", ALU.bypass, replica_groups=rg,
                    ins=[warm_in.opt()], outs=[warm_out.opt()])

            # ---- Stage A interleaved per chunk:
            #   x1^T = transpose(x[:, 0::2]);  h0^T = Win^T @ x1^T + bin ----
            win = [wp.tile([128, H], bf16, tag=f"w{k}", name=f"wtin_{k}")
                   for k in range(LT)]
            binT = sp.tile([128, MT], f32, tag="biasT")

            x1 = [hp.tile([128, C], bf16, tag=f"xt{j}", name=f"x1_{j}")
                  for j in range(LT)]
            if LREM < 128:
                # zero the padded partitions once (whole tile: partition
                # slices must be 32-aligned; real rows overwritten below).
                # On DVE: the GpSimd queue is serialized behind the warmup
                # collectives' pool-rotation waits and would gate stage A.
                nc.vector.memset(x1[LT - 1][:], 0.0)
            cur = [hp.tile([128, C], bf16, tag=f"hg{m}", name=f"h0_{m}")
                   for m in range(MT)]
            hblT = sp.tile([128, MT], f32, tag="biasL", name="bh_l0")
            gT = sp.tile([128, MT], f32, tag="gT", name="g_l0")
            bT = sp.tile([128, MT], f32, tag="bT", name="b_l0")
            for n in range(NCH):
                ncs = slice(n * NCHW, (n + 1) * NCHW)
                for b in range(n * BPC, (n + 1) * BPC):
                    xin = xp.tile([128, D], f32, tag="xin", bufs=5)
                    if b == 0:
                        # split the first tile across 4 HWDGE queues so the
                        # input chain starts earlier
                        for q in range(4):
                            nc.sync.dma_start(
                                xin[q * 32:(q + 1) * 32, :],
                                x_d[b * 128 + q * 32:b * 128 + (q + 1) * 32, :])
                    else:
                        nc.sync.dma_start(xin[:], x_d[b * 128:(b + 1) * 128, :])
                    xe = xp.tile([128, L], bf16, tag="xe", bufs=3)
                    nc.scalar.copy(
                        xe[:],
                        xin[:].rearrange("p (l two) -> p l two", two=2)[:, :, 0])
                    for j in range(LT):
                        wj = 128 if j < LT - 1 else LREM
                        ps = pp.tile([128, 128], bf16, tag="tr", bufs=2)
                        nc.tensor.transpose(ps[0:wj, :],
                                            xe[:, j * 128:j * 128 + wj],
                                            ident[:])
                        nc.vector.tensor_copy(
                            x1[j][0:wj, b * 128:(b + 1) * 128], ps[0:wj, :])
                if n == 0:
                    # Win/bin DMAs issued after chunk-0's x tiles so the
                    # input chain isn't stuck behind weight traffic
                    for k in range(LT):
                        nc.sync.dma_start(win[k][:],
                                          win_d[k * 128:(k + 1) * 128, :])
                    nc.sync.dma_start(
                        binT[:], bin_d[:].rearrange("(m p) -> p m", p=128))
                if n == NCH - 1:
                    # layer 0's weights + bn params, after the last x tiles
                    # so they don't delay the stage-A input pipeline
                    nc.sync.dma_start(hblT[:],
                                      bh_d[0, :].rearrange("(m p) -> p m",
                                                           p=128))
                    nc.sync.dma_start(gT[:],
                                      gamma_d[0, :].rearrange("(m p) -> p m",
                                                              p=128))
                    nc.sync.dma_start(bT[:],
                                      beta_d[0, :].rearrange("(m p) -> p m",
                                                             p=128))
                for m in range(MT):
                    ps = pp.tile([128, NCHW], f32, tag="mm")
                    for k in range(LT):
                        nc.tensor.matmul(ps[:], win[k][:, m * 128:(m + 1) * 128],
                                         x1[k][:, ncs],
                                         start=(k == 0), stop=(k == LT - 1))
                    # bias-add drain alternates DVE/ACT so PSUM slots recycle
                    # at PE rate; output is the bf16 moving operand of layer 0
                    if m % 2 == 0:
                        nc.vector.tensor_scalar(
                            out=cur[m][:, ncs], in0=ps[:],
                            scalar1=binT[:, m:m + 1], scalar2=None, op0=ALU.add)
                    else:
                        nc.scalar.activation(
                            cur[m][:, ncs], ps[:], AF.Identity,
                            bias=binT[:, m:m + 1], scale=1.0)
                if n == NCH - 1:
                    wt = [wp.tile([128, H], bf16, tag=f"w{k}",
                                  name=f"wh_0_{k}") for k in range(MT)]
                    for k in range(MT):
                        nc.sync.dma_start(wt[k][:],
                                          wh_d[0, k * 128:(k + 1) * 128, :])

            # ---- Hidden blocks: h = BN(relu(Wh^T @ h + bh)) ----
            # The per-layer BN sync is software-pipelined around the ~12us
            # AllGather flight.  Stats for m=0..4 ship in AG#1 (launched
            # mid-m-loop, landing before the loop ends); m=5..7 in AG#2
            # (launched right after m=7's stats).  The consumer hides AG#2's
            # flight with deferred-k accumulation: the first SPILL_M m-tiles
            # accumulate k=0..4 into PSUM, spill the partial to SBUF (freeing
            # the bank), and only later add k=5..7 once AG#2's normalize is
            # out.  That yields ~21us of PE work that needs no AG#2 data.
            MSPLIT = 5
            SPILL_M = 4

            def bn_launch(tag, sl, mean_ap, var_ap, l):
                # pack local (mean, var) -> (sum, sumsq)/B; AllReduce sums
                # them across cores.  (AllGather + on-chip reduce needs a
                # strided 56B-segment gather-back DMA that costs ~6us.)
                w = sl.stop - sl.start
                sums = sp.tile([128, 2 * w], f32, tag=f"sums{tag}",
                               name=f"sums{tag}_{l}")
                nc.vector.tensor_scalar_mul(sums[:, 0:w], mean_ap[:, sl],
                                            float(C) / B)
                msq = sp.tile([128, w], f32, tag=f"msq{tag}",
                               name=f"msq{tag}_{l}")
                nc.vector.tensor_mul(msq[:], mean_ap[:, sl], mean_ap[:, sl])
                nc.vector.tensor_add(sums[:, w:2 * w], var_ap[:, sl], msq[:])
                nc.vector.tensor_scalar_mul(sums[:, w:2 * w],
                                            sums[:, w:2 * w], float(C) / B)
                agin = dp.tile([128, 2 * w], f32, tag=f"arin{tag}",
                               name=f"arin{tag}_{l}")
                agout = dp.tile([n_cores * 128, 2 * w], f32, tag=f"arout{tag}",
                                name=f"arout{tag}_{l}", addr_space="Shared")
                nc.gpsimd.dma_start(agin[:], sums[:])
                # AllGather, not AllReduce: the ring AR costs ~2x the steps
                # (~19us vs ~6us measured for this tiny payload)
                nc.gpsimd.collective_compute(
                    "AllGather", ALU.bypass, replica_groups=rg,
                    ins=[agin.opt()], outs=[agout.opt()])
                return agout

            def bn_gather(tag, agout, sl, l):
                # gather-back is DMA-segment-rate bound (128 tiny segments
                # per rank): split per-rank over GpSimd+scalar queues.
                # Emitted only where those queues have no pending drain
                # work, since the DMAs wait on the collective.
                w = sl.stop - sl.start
                gall = sp.tile([128, n_cores * 2 * w], f32, tag=f"gall{tag}",
                               name=f"gall{tag}_{l}")
                engs = [nc.gpsimd, nc.scalar] * (n_cores // 2)
                for r in range(n_cores):
                    engs[r].dma_start(
                        gall[:, r * 2 * w:(r + 1) * 2 * w],
                        agout[r * 128:(r + 1) * 128, :])
                return gall

            def bn_finish(tag, sl, gall, gT, bT, aa, bb, l, eng):
                # reduce over ranks, then a = gamma*rsqrt(var+eps),
                # b = beta - mean*a.  `eng` picks the vector engine so this
                # chain stays off the queue that m=7's stats and AG#2's
                # pack need.
                w = sl.stop - sl.start
                gst = sp.tile([128, 2 * w], f32, tag=f"gst{tag}",
                              name=f"gst{tag}_{l}")
                nc.vector.tensor_reduce(
                    gst[:],
                    gall[:].rearrange("p (r s) -> p s r", s=2 * w),
                    axis=AX.X, op=ALU.add)
                gm = gst[:, 0:w]
                ge2 = gst[:, w:2 * w]
                gve = sp.tile([128, w], f32, tag=f"gve{tag}",
                              name=f"gve{tag}_{l}")
                eng.tensor_mul(gve[:], gm, gm)
                eng.tensor_sub(gve[:], ge2, gve[:])
                gstd = sp.tile([128, w], f32, tag=f"gstd{tag}",
                               name=f"gstd{tag}_{l}")
                nc.scalar.activation(gstd[:], gve[:], AF.Sqrt,
                                     bias=eps_t[:, 0:1], scale=1.0)
                ginv = sp.tile([128, w], f32, tag=f"ginv{tag}",
                               name=f"ginv{tag}_{l}")
                nc.vector.reciprocal(ginv[:], gstd[:])
                eng.tensor_mul(aa[:], gT[:, sl], ginv[:])
                mb = sp.tile([128, w], f32, tag=f"mb{tag}",
                             name=f"mb{tag}_{l}")
                eng.tensor_mul(mb[:], gm, aa[:])
                eng.tensor_sub(bb[:], bT[:, sl], mb[:])

            SL1 = slice(0, MSPLIT)
            SL2 = slice(MSPLIT, MT)
            wo = None
            boutrow = None
            xin2 = [None] * (C // 128)
            for l in range(NL):
                bhT = hblT
                outt = [hp.tile([128, C], bf16, tag=f"hr{m}",
                                name=f"hp_{l}_{m}") for m in range(MT)]
                ag = sp.tile([128, 2 * MT], f32, tag="ag")
                mean_ap = ag[:].rearrange("p (m two) -> p m two", two=2)[:, :, 0]
                var_ap = ag[:].rearrange("p (m two) -> p m two", two=2)[:, :, 1]
                aa1 = sp.tile([128, MSPLIT], f32, tag="aa1")
                bb1 = sp.tile([128, MSPLIT], f32, tag="bb1")
                aa2 = sp.tile([128, MT - MSPLIT], f32, tag="aa2")
                bb2 = sp.tile([128, MT - MSPLIT], f32, tag="bb2")

                # prefetches for the next stage ride the sync queue early
                if l < NL - 1:
                    wtn = [wp.tile([128, H], bf16, tag=f"w{k}",
                                   name=f"wh_{l + 1}_{k}")
                           for k in range(MT)]
                    for k in range(MT):
                        nc.sync.dma_start(
                            wtn[k][:],
                            wh_d[l + 1, k * 128:(k + 1) * 128, :])
                    hblT = sp.tile([128, MT], f32, tag="biasL",
                                   name=f"bh_l{l + 1}")
                    nc.sync.dma_start(
                        hblT[:],
                        bh_d[l + 1, :].rearrange("(m p) -> p m", p=128))
                    gTn = sp.tile([128, MT], f32, tag="gT",
                                  name=f"g_l{l + 1}")
                    nc.sync.dma_start(
                        gTn[:],
                        gamma_d[l + 1, :].rearrange("(m p) -> p m", p=128))
                    bTn = sp.tile([128, MT], f32, tag="bT",
                                  name=f"b_l{l + 1}")
                    nc.sync.dma_start(
                        bTn[:],
                        beta_d[l + 1, :].rearrange("(m p) -> p m", p=128))
                else:
                    wo = [wp.tile([128, L], bf16, tag=f"wo{k}",
                                  bufs=1, name=f"wtout_{k}")
                          for k in range(MT)]
                    for k in range(MT):
                        nc.sync.dma_start(
                            wo[k][:], wout_d[k * 128:(k + 1) * 128, :])
                    boutrow = sp.tile([1, L], bf16, tag="boutT")
                    nc.sync.dma_start(boutrow[:], bout_d[:, :])
                    for b in range(C // 128):
                        xin2[b] = xp.tile([128, D], f32, tag="xin2",
                                          bufs=8, name=f"xin2_{b}")
                        nc.sync.dma_start(
                            xin2[b][:], x_d[b * 128:(b + 1) * 128, :])

                def mm_block(m, k0, k1, l=l):
                    # one accumulation group over k-tiles [k0, k1) x 4 chunks
                    pss = [pp.tile([128, NCHW], f32, tag="mm",
                                   name=f"ps_{l}_{m}_{k0}_{n}")
                           for n in range(NCH)]
                    for k in range(k0, k1):
                        for n in range(NCH):
                            nc.tensor.matmul(
                                pss[n][:], wt[k][:, m * 128:(m + 1) * 128],
                                cur[k][:, n * NCHW:(n + 1) * NCHW],
                                start=(k == k0), stop=(k == k1 - 1))
                    return pss

                def stats_block(m, l=l):
                    st = sp.tile([128, 6 * NCH], f32, tag="st")
                    for n in range(NCH):
                        ncs = slice(n * NCHW, (n + 1) * NCHW)
                        nc.vector.bn_stats(st[:, 6 * n:6 * n + 6],
                                           outt[m][:, ncs])
                    nc.vector.bn_aggr(ag[:, 2 * m:2 * m + 2], st[:])

                spill_m = SPILL_M if l > 0 else 0
                scr = [hp.tile([128, C], bf16,
                               tag=(f"xt{m}" if m < 4 else f"sc{m}"),
                               name=f"sc_{l}_{m}") for m in range(spill_m)]
                # phase A: k=0..MSPLIT-1 accumulate, spill partial to SBUF
                for m in range(spill_m):
                    pss = mm_block(m, 0, MSPLIT)
                    for n in range(NCH):
                        ncs = slice(n * NCHW, (n + 1) * NCHW)
                        nc.vector.tensor_copy(scr[m][:, ncs], pss[n][:])
                # phase B: re-inject the spilled partial through the PE (an
                # identity matmul seeds the accumulation — keeps the merge
                # off DVE, whose stats work would otherwise pace the PE),
                # then k=MSPLIT..MT-1 accumulate and a normal relu drain.
                for m in range(spill_m):
                    pss = [pp.tile([128, NCHW], f32, tag="mm",
                                   name=f"psb_{l}_{m}_{n}")
                           for n in range(NCH)]
                    for n in range(NCH):
                        ncs = slice(n * NCHW, (n + 1) * NCHW)
                        nc.tensor.matmul(pss[n][:], ident[:],
                                         scr[m][:, ncs],
                                         start=True, stop=False)
                    for k in range(MSPLIT, MT):
                        for n in range(NCH):
                            nc.tensor.matmul(
                                pss[n][:], wt[k][:, m * 128:(m + 1) * 128],
                                cur[k][:, n * NCHW:(n + 1) * NCHW],
                                start=False, stop=(k == MT - 1))
                    for n in range(NCH):
                        ncs = slice(n * NCHW, (n + 1) * NCHW)
                        nc.scalar.activation(outt[m][:, ncs], pss[n][:],
                                             AF.Relu, bias=bhT[:, m:m + 1],
                                             scale=1.0)
                    stats_block(m)
                    if m == MSPLIT - 1:
                        # AG#1 (m0..3) launches as soon as its stats exist
                        ag1out = bn_launch("1", SL1, mean_ap, var_ap, l)
                # remaining m-tiles: single full-k accumulation
                for m in range(spill_m, MT):
                    pss = mm_block(m, 0, MT)
                    for n in range(NCH):
                        ncs = slice(n * NCHW, (n + 1) * NCHW)
                        if m == MT - 1:
                            # last tile drains on DVE so the ACT queue is
                            # free the moment the m-loop ends (it holds the
                            # early normalize chunks and finish#1's sqrt)
                            nc.vector.tensor_scalar(
                                out=outt[m][:, ncs], in0=pss[n][:],
                                scalar1=bhT[:, m:m + 1], scalar2=0.0,
                                op0=ALU.add, op1=ALU.max)
                        else:
                            nc.scalar.activation(outt[m][:, ncs], pss[n][:],
                                                 AF.Relu, bias=bhT[:, m:m + 1],
                                                 scale=1.0)
                    stats_block(m)
                    if m == MSPLIT - 1:
                        # AG#1 (m0..4) flies while m5..7 compute
                        ag1out = bn_launch("1", SL1, mean_ap, var_ap, l)
                    if m == MT - 2:
                        # gather + a,b chain emitted here: they run in the
                        # queue-idle window before the m-loop ends (m7's
                        # drains are on DVE; ACT is free after m6's)
                        gall1 = bn_gather("1", ag1out, SL1, l)
                        bn_finish("1", SL1, gall1, gT, bT, aa1, bb1, l,
                                  nc.gpsimd)

                def norm_chunk(k, n, ei, nxt):
                    ncs = slice(n * NCHW, (n + 1) * NCHW)
                    aa, bb, j = ((aa1, bb1, k) if k < MSPLIT
                                 else (aa2, bb2, k - MSPLIT))
                    if ei == 0:
                        nc.vector.tensor_scalar(
                            out=nxt[k][:, ncs], in0=outt[k][:, ncs],
                            scalar1=aa[:, j:j + 1], scalar2=bb[:, j:j + 1],
                            op0=ALU.mult, op1=ALU.add)
                    elif ei == 1:
                        nc.scalar.activation(
                            nxt[k][:, ncs], outt[k][:, ncs], AF.Identity,
                            bias=bb[:, j:j + 1], scale=aa[:, j:j + 1])
                    else:
                        nc.gpsimd.tensor_scalar(
                            out=nxt[k][:, ncs], in0=outt[k][:, ncs],
                            scalar1=aa[:, j:j + 1], scalar2=bb[:, j:j + 1],
                            op0=ALU.mult, op1=ALU.add)

                # normalize k=0..4 immediately (a,b landed mid-loop).
                # Hidden layers consume whole k-rows (k-major); the output
                # stage consumes column-blocks (n-major).  Lead with
                # GpSimd/ACT — DVE still holds m7's drains+stats.
                nxt = [hp.tile([128, C], bf16, tag=f"hg{m}",
                               name=f"hn_{l}_{m}") for m in range(MT)]
                if l < NL - 1:
                    order = [(k, n) for k in range(MSPLIT)
                             for n in range(NCH)]
                else:
                    order = [(k, n) for n in range(NCH)
                             for k in range(MSPLIT)]
                for idx, (k, n) in enumerate(order):
                    ei = (2 if idx % 2 == 0 else 1) if idx < 8 else idx % 3
                    norm_chunk(k, n, ei, nxt)
                # AG#2 (m5..7): pack rides DVE after m7's stats; its tail
                # lands under the next consumer's deferred-k cover
                ag2out = bn_launch("2", SL2, mean_ap, var_ap, l)
                gall2 = bn_gather("2", ag2out, SL2, l)
                bn_finish("2", SL2, gall2, gT, bT, aa2, bb2, l, nc.vector)
                ei = 0
                for k in range(MSPLIT, MT):
                    for n in range(NCH):
                        norm_chunk(k, n, ei, nxt)
                        ei = (ei + 1) % 3
                cur = nxt
                if l < NL - 1:
                    wt = wtn
                    gT = gTn
                    bT = bTn

            # ---- Output stage:  y = h^T-blocks (stationary) @ Wout + bout;
            #   out[:,0::2] = x1, out[:,1::2] = x2 + y  (no PE transposes).
            # Same A/B deferral: every block's bout + k0..4 partial runs
            # while layer 4's AG#2 is in flight, spilled to SBUF; k5..7 and
            # the assembly follow once the last normalize lands. ----
            scro = []
            for b in range(C // 128):
                bs = slice(b * 128, (b + 1) * 128)
                ps = pp.tile([128, NCHW], f32, tag="mm", name=f"oA_{b}")
                # rank-1: seed every row of the PSUM group with bout
                nc.tensor.matmul(ps[:, 0:L], ones[:], boutrow[:],
                                 start=True, stop=False)
                for k in range(MSPLIT):
                    nc.tensor.matmul(ps[:, 0:L], cur[k][:, bs], wo[k][:],
                                     start=False, stop=(k == MSPLIT - 1))
                so = xp.tile([128, L], bf16, tag="so", bufs=16,
                             name=f"so_{b}")
                nc.vector.tensor_copy(so[:], ps[:, 0:L])
                scro.append(so)
            for b in range(C // 128):
                bs = slice(b * 128, (b + 1) * 128)
                ps = pp.tile([128, NCHW], f32, tag="mm", name=f"oB_{b}")
                # identity matmul re-injects the spilled partial, then k5..7
                nc.tensor.matmul(ps[:, 0:L], ident[:], scro[b][:],
                                 start=True, stop=False)
                for k in range(MSPLIT, MT):
                    nc.tensor.matmul(ps[:, 0:L], cur[k][:, bs], wo[k][:],
                                     start=False, stop=(k == MT - 1))
                xo = xp.tile([128, D], f32, tag="xo", bufs=4)
                xin_il = xin2[b][:].rearrange("p (l two) -> p l two", two=2)
                xo_il = xo[:].rearrange("p (l two) -> p l two", two=2)
                nc.scalar.copy(xo_il[:, :, 0], xin_il[:, :, 0])
                # balance the tail pipeline: odd-add alternates DVE/GpSimd
                # (GpSimd can't read PSUM, so it gets a bounce via DVE),
                # output DMA alternates sync/scalar queues
                nc.vector.tensor_add(xo_il[:, :, 1], ps[:, 0:L],
                                     xin_il[:, :, 1])
                if b % 2 == 0:
                    nc.sync.dma_start(out_d[bs, :], xo[:])
                else:
                    nc.scalar.dma_start(out_d[bs, :], xo[:])

    nc.compile()
    return nc


def make_in_maps(x, Win, bin_, Wh, bh, gamma, beta, Wout, bout,
                 B=B_FULL, D=D_FULL, H=H_FULL, n_cores=NCORES):
    import ml_dtypes

    bf16 = ml_dtypes.bfloat16
    L = D // 2
    C = B // n_cores
    LT = (L + 127) // 128
    x = np.ascontiguousarray(np.asarray(x, dtype=np.float32))
    win_p = np.zeros((LT * 128, H), dtype=bf16)
    win_p[:L] = np.asarray(Win, dtype=np.float32).astype(bf16)
    common = {
        "win": win_p,
        "wh": np.ascontiguousarray(np.asarray(Wh, dtype=np.float32)
                                   .astype(bf16)),
        "wout": np.ascontiguousarray(np.asarray(Wout, dtype=np.float32)
                                     .astype(bf16)),
        "bin": np.asarray(bin_, dtype=np.float32),
        "bh": np.ascontiguousarray(np.asarray(bh, dtype=np.float32)),
        "gamma": np.ascontiguousarray(np.asarray(gamma, dtype=np.float32)),
        "beta": np.ascontiguousarray(np.asarray(beta, dtype=np.float32)),
        "bout": np.asarray(bout, dtype=np.float32).astype(bf16)
                .reshape(1, L),
    }
    return [
        {"x": np.ascontiguousarray(x[c * C:(c + 1) * C]), **common}
        for c in range(n_cores)
    ]


_built = None


def kernel(x, Win, bin_, Wh, bh, gamma, beta, Wout, bout):
    global _built
    from concourse.bass_utils import run_bass_kernel_spmd

    if _built is None:
        _built = build_kernel()
    in_maps = make_in_maps(x, Win, bin_, Wh, bh, gamma, beta, Wout, bout)
    res = run_bass_kernel_spmd(_built, in_maps, core_ids=list(range(NCORES)))
    return np.concatenate([r["out"] for r in res.results], axis=0)
